# revision 19
# baseline (speedup 1.0000x reference)
"""TRN2 Bass kernel for nn_EnsemblePointNet: 1296 independent 4-layer MLPs.

Strategy: shard the model dim (1296 -> 162 per core) across 8 NeuronCores.
Per model, activations live transposed ([feature, batch]) so every layer is
one PE matmul with the stored weights as lhsT:
    z = W.T @ h   (lhsT=W [K,M], rhs=h_T [K,B])
fp32r matmuls (full PE rate, ~1.5e-4 rel err). Bias+ReLU fused into the
PSUM->SBUF eviction pass, split between the Scalar (ACT) and Vector (DVE)
engines. DMAs are batched across models (per-DMA fixed cost ~1us dominates
otherwise). The [1,B] layer-3 outputs are evicted to a partition-0 scratch
row shared by XGRP models, DMA'd into a 128-model gather tile, bias-added
there, and stored contiguously.
"""

import contextlib
import sys

sys.path.insert(0, "/opt/trn_rl_repo")

import numpy as np

import concourse.bass as bass
import concourse.mybir as mybir
import concourse.tile as tile
from concourse import bacc
from concourse.bass_utils import run_bass_kernel_spmd

F32 = mybir.dt.float32
F32R = mybir.dt.float32r
F16 = mybir.dt.float16
AF = mybir.ActivationFunctionType
OP = mybir.AluOpType

M_TOT = 1296
N_CORES = 8
M_LOC = M_TOT // N_CORES  # 162
B = 1024
DIN = 8
H = 128

# ACT is 1.2 GHz, DVE 0.96 GHz; split each relu pass proportionally.
ACT_COLS = 640  # of 1024
HB = 512  # psum bank width in fp32

WGRP = 3   # models per w12 load (w1+w2 packed, 128KB each)
XGRP = 6   # models per xt load / scr evict row
W0GRP = 18  # models per w0 load
GRP = 54   # models per output gather group
ILV = 3    # models interleaved per pipeline round (col-tiled L3 trios)


def build_nc(m_loc=M_LOC, loop_n=1):
    assert m_loc % GRP == 0 and GRP % XGRP == 0 and GRP % WGRP == 0 and GRP % W0GRP == 0
    nc = bacc.Bacc("TRN2", target_bir_lowering=False, debug=False)
    xt = nc.dram_tensor("xt", [m_loc, DIN, B], F32, kind="ExternalInput").ap()
    w0 = nc.dram_tensor("w0", [m_loc, DIN, H], F32, kind="ExternalInput").ap()
    w12 = nc.dram_tensor("w12", [m_loc, 2, H, H], F32, kind="ExternalInput").ap()
    w3t16 = nc.dram_tensor("w3t16", [H, m_loc + 31], F16, kind="ExternalInput").ap()
    b0t = nc.dram_tensor("b0t", [H, m_loc], F32, kind="ExternalInput").ap()
    b1t = nc.dram_tensor("b1t", [H, m_loc], F32, kind="ExternalInput").ap()
    b2t = nc.dram_tensor("b2t", [H, m_loc], F32, kind="ExternalInput").ap()
    ngrp = m_loc // GRP
    b3t = nc.dram_tensor("b3t", [GRP, ngrp], F32, kind="ExternalInput").ap()
    y = nc.dram_tensor("y", [m_loc, B], F32, kind="ExternalOutput").ap()

    with tile.TileContext(nc) as tc:
        with (
            tc.tile_pool(name="consts", bufs=1) as consts,
            tc.tile_pool(name="wpool", bufs=4) as wpool,
            tc.tile_pool(name="w0pool", bufs=2) as w0pool,
            tc.tile_pool(name="xpool", bufs=3) as xpool,
            tc.tile_pool(name="hpool", bufs=5) as hpool,
            tc.tile_pool(name="ypool", bufs=2) as ypool,
            tc.tile_pool(name="spool", bufs=2) as spool,
            tc.tile_pool(name="zpool", bufs=6, space="PSUM") as zpool,
            tc.tile_pool(name="zqpool", bufs=2, space="PSUM") as zqpool,
        ):
            # one-time constants
            w3t_s = consts.tile([H, m_loc + 31], F16)
            nc.sync.dma_start(out=w3t_s, in_=w3t16)
            b0t_s = consts.tile([H, m_loc], F32)
            nc.sync.dma_start(out=b0t_s, in_=b0t)
            b1t_s = consts.tile([H, m_loc], F32)
            nc.sync.dma_start(out=b1t_s, in_=b1t)
            b2t_s = consts.tile([H, m_loc], F32)
            nc.sync.dma_start(out=b2t_s, in_=b2t)
            b3t_s = consts.tile([GRP, ngrp], F32)
            nc.sync.dma_start(out=b3t_s, in_=b3t)

            def relu_half(dst_half, z_half, bias_ap, on_act):
                # dst[128, HB] sbuf <- relu(z[128, HB] psum + bias)
                if on_act:
                    nc.scalar.activation(
                        dst_half, z_half, AF.Relu, bias=bias_ap, scale=1.0,
                    )
                else:
                    nc.vector.tensor_scalar(
                        out=dst_half, in0=z_half,
                        scalar1=bias_ap, scalar2=0.0, op0=OP.add, op1=OP.max,
                    )

            def body():
                w12s = xts = w0s = scrw = ygat = None

                def load_batches(m):
                    nonlocal w12s, xts, w0s, scrw
                    if m % W0GRP == 0:
                        w0s = w0pool.tile([DIN, W0GRP * H], F32R, tag="w0")
                        nc.sync.dma_start(
                            out=w0s,
                            in_=w0[m : m + W0GRP].rearrange("m i h -> i m h").bitcast(F32R),
                        )
                    if m % WGRP == 0:
                        w12s = wpool.tile([H, WGRP * 2 * H], F32R, tag="w12")
                        nc.sync.dma_start(
                            out=w12s,
                            in_=w12[m : m + WGRP].rearrange("m l h k -> h m l k").bitcast(F32R),
                        )
                    if m % XGRP == 0:
                        xts = xpool.tile([DIN, XGRP * B], F32R, tag="xt")
                        nc.sync.dma_start(
                            out=xts,
                            in_=xt[m : m + XGRP].rearrange("m i b -> i m b").bitcast(F32R),
                        )

                for g in range(ngrp):
                    g0 = g * GRP
                    ygat = ypool.tile([GRP, B], F32, tag="ygat")
                    rounds = [list(range(g0 + r, min(g0 + r + ILV, g0 + GRP)))
                              for r in range(0, GRP, ILV)]
                    for models in rounds:
                        ctx = {}
                        for m in models:
                            load_batches(m)
                            ctx[m] = (w12s, xts, w0s,
                                      (m % XGRP) * B, (m % WGRP) * 2 * H, (m % W0GRP) * H)

                        def mm_layer(lsel, rhs_of, zdict):
                            for m in models:
                                ws_, xs_, w0_, xo, wo, w0o = ctx[m]
                                za = zpool.tile([H, HB], F32, tag="z")
                                zb = zpool.tile([H, HB], F32, tag="z")
                                if lsel == 0:
                                    lhs = w0_[:, w0o : w0o + H]
                                    ra = xs_[:, xo : xo + HB]
                                    rb = xs_[:, xo + HB : xo + B]
                                else:
                                    lhs = ws_[:, wo + (lsel - 1) * H : wo + lsel * H]
                                    h = rhs_of[m]
                                    ra = h[:, 0:HB]
                                    rb = h[:, HB:B]
                                nc.tensor.matmul(za, lhs, ra, start=True, stop=True)
                                nc.tensor.matmul(zb, lhs, rb, start=True, stop=True)
                                zdict[m] = (za, zb)

                        def relu_layer(zdict, bias_t, tag, dt, hdict):
                            for m in models:
                                za, zb = zdict[m]
                                h = hpool.tile([H, B], dt, tag=tag)
                                relu_half(h[:, 0:HB], za, bias_t[:, m : m + 1], True)
                                relu_half(h[:, HB:B], zb, bias_t[:, m : m + 1], False)
                                hdict[m] = h

                        zs, hs = {}, {}
                        mm_layer(0, None, zs)
                        relu_layer(zs, b0t_s, "h1", F32R, hs)
                        zs = {}
                        mm_layer(1, hs, zs)
                        h2s = {}
                        relu_layer(zs, b1t_s, "h2", F32R, h2s)
                        zs = {}
                        mm_layer(2, h2s, zs)
                        h3s = {}
                        relu_layer(zs, b2t_s, "h3", F16, h3s)

                        # col-tiled quad L3: model j -> psum rows 32j
                        zqa = zqpool.tile([128, HB], F32, tag="zq")
                        zqb = zqpool.tile([128, HB], F32, tag="zq")
                        for j, m in enumerate(models):
                            nc.tensor.matmul(
                                zqa[32 * j : 32 * j + 32, :],
                                w3t_s[:, m : m + 32], h3s[m][:, 0:HB],
                                start=True, stop=True, tile_position=(0, 32 * j),
                            )
                            nc.tensor.matmul(
                                zqb[32 * j : 32 * j + 32, :],
                                w3t_s[:, m : m + 32], h3s[m][:, HB:B],
                                start=True, stop=True, tile_position=(0, 32 * j),
                            )
                        scr = spool.tile([128, B], F32, tag="scr")
                        nc.scalar.copy(out=scr[:, 0:HB], in_=zqa)
                        nc.scalar.copy(out=scr[:, HB:B], in_=zqb)
                        mi0 = models[0] - g0
                        nq = len(models)
                        sv = scr.rearrange("(a p) b -> a p b", a=4)[0:nq, 0, :]
                        nc.sync.dma_start(out=ygat[mi0 : mi0 + nq, :], in_=sv)

                    # bias add + store for the group
                    yout = ypool.tile([GRP, B], F32, tag="yout")
                    nc.scalar.add(yout[0:GRP], ygat[0:GRP], b3t_s[0:GRP, g : g + 1])
                    nc.sync.dma_start(out=y[g0 : g0 + GRP, :], in_=yout[0:GRP])

            if loop_n > 1:
                with tc.For_i(0, loop_n, 1):
                    body()
            else:
                body()

    nc.compile()
    return nc


_NC_CACHE = {}


def _get_nc(m_loc):
    if m_loc not in _NC_CACHE:
        _NC_CACHE[m_loc] = build_nc(m_loc)
    return _NC_CACHE[m_loc]


def _prep_core_inputs(x, W0, b0, W1, b1, W2, b2, W3, b3, sl):
    m_loc = sl.stop - sl.start
    ngrp = m_loc // GRP
    xt = np.ascontiguousarray(np.transpose(x[sl], (0, 2, 1)))  # [m, DIN, B]
    w12 = np.ascontiguousarray(
        np.stack([W1[sl], W2[sl]], axis=1)  # [m, 2, H, H]
    )
    b3_pad = b3[sl, 0].astype(np.float32)
    return {
        "xt": xt,
        "w0": np.ascontiguousarray(W0[sl]),
        "w12": w12,
        "w3t16": np.ascontiguousarray(
            np.pad(W3[sl, :, 0], ((0, 31), (0, 0))).T.astype(np.float16)
        ),  # [H, m+31]
        "b0t": np.ascontiguousarray(b0[sl].T),
        "b1t": np.ascontiguousarray(b1[sl].T),
        "b2t": np.ascontiguousarray(b2[sl].T),
        "b3t": np.ascontiguousarray(b3_pad.reshape(ngrp, GRP).T),
    }


def kernel(x, W0, b0, W1, b1, W2, b2, W3, b3):
    x = np.asarray(x, dtype=np.float32)
    W0 = np.asarray(W0, np.float32); b0 = np.asarray(b0, np.float32)
    W1 = np.asarray(W1, np.float32); b1 = np.asarray(b1, np.float32)
    W2 = np.asarray(W2, np.float32); b2 = np.asarray(b2, np.float32)
    W3 = np.asarray(W3, np.float32); b3 = np.asarray(b3, np.float32)

    m_tot = x.shape[0]
    m_loc = m_tot // N_CORES
    nc = _get_nc(m_loc)
    in_maps = [
        _prep_core_inputs(x, W0, b0, W1, b1, W2, b2, W3, b3,
                          slice(c * m_loc, (c + 1) * m_loc))
        for c in range(N_CORES)
    ]
    res = run_bass_kernel_spmd(nc, in_maps, core_ids=list(range(N_CORES)))
    out = np.concatenate([r["y"] for r in res.results], axis=0)
    return out.reshape(m_tot, B, 1).astype(np.float32)


# revision 21
# speedup vs baseline: 1.0450x; 1.0450x over previous
"""TRN2 Bass kernel for nn_EnsemblePointNet: 1296 independent 4-layer MLPs.

Strategy: shard the model dim (1296 -> 162 per core) across 8 NeuronCores.
Per model, activations live transposed ([feature, batch]) so every layer is
one PE matmul with the stored weights as lhsT:
    z = W.T @ h   (lhsT=W [K,M], rhs=h_T [K,B])
fp32r matmuls (full PE rate, ~1.5e-4 rel err). Bias+ReLU fused into the
PSUM->SBUF eviction pass, split between the Scalar (ACT) and Vector (DVE)
engines. DMAs are batched across models (per-DMA fixed cost ~1us dominates
otherwise). The [1,B] layer-3 outputs are evicted to a partition-0 scratch
row shared by XGRP models, DMA'd into a 128-model gather tile, bias-added
there, and stored contiguously.
"""

import contextlib
import sys

sys.path.insert(0, "/opt/trn_rl_repo")

import numpy as np

import concourse.bass as bass
import concourse.mybir as mybir
import concourse.tile as tile
from concourse import bacc
from concourse.bass_utils import run_bass_kernel_spmd

F32 = mybir.dt.float32
F32R = mybir.dt.float32r
F16 = mybir.dt.float16
AF = mybir.ActivationFunctionType
OP = mybir.AluOpType

M_TOT = 1296
N_CORES = 8
M_LOC = M_TOT // N_CORES  # 162
B = 1024
DIN = 8
H = 128

# ACT is 1.2 GHz, DVE 0.96 GHz; split each relu pass proportionally.
ACT_COLS = 640  # of 1024
HB = 512  # psum bank width in fp32

WGRP = 3   # models per w12 load (w1+w2 packed, 128KB each)
XGRP = 6   # models per xt load / scr evict row
W0GRP = 18  # models per w0 load
GRP = 54   # models per output gather group
ILV = 4    # models interleaved per pipeline round (quad for col-tiled L3)


def build_nc(m_loc=M_LOC, loop_n=1, do_pass=True, do_mm=True, do_dma=True):
    assert m_loc % GRP == 0 and GRP % XGRP == 0 and GRP % WGRP == 0 and GRP % W0GRP == 0
    nc = bacc.Bacc("TRN2", target_bir_lowering=False, debug=False)
    xt = nc.dram_tensor("xt", [m_loc, DIN, B], F32, kind="ExternalInput").ap()
    w0 = nc.dram_tensor("w0", [m_loc, DIN, H], F32, kind="ExternalInput").ap()
    w12 = nc.dram_tensor("w12", [m_loc, 2, H, H], F32, kind="ExternalInput").ap()
    w3t16 = nc.dram_tensor("w3t16", [H, m_loc + 31], F16, kind="ExternalInput").ap()
    b0t = nc.dram_tensor("b0t", [H, m_loc], F32, kind="ExternalInput").ap()
    b1t = nc.dram_tensor("b1t", [H, m_loc], F32, kind="ExternalInput").ap()
    b2t = nc.dram_tensor("b2t", [H, m_loc], F32, kind="ExternalInput").ap()
    ngrp = m_loc // GRP
    b3t = nc.dram_tensor("b3t", [GRP, ngrp], F32, kind="ExternalInput").ap()
    y = nc.dram_tensor("y", [m_loc, B], F32, kind="ExternalOutput").ap()

    with tile.TileContext(nc) as tc:
        with (
            tc.tile_pool(name="consts", bufs=1) as consts,
            tc.tile_pool(name="wpool", bufs=4) as wpool,
            tc.tile_pool(name="w0pool", bufs=2) as w0pool,
            tc.tile_pool(name="xpool", bufs=3) as xpool,
            tc.tile_pool(name="hpool", bufs=6) as hpool,
            tc.tile_pool(name="ypool", bufs=2) as ypool,
            tc.tile_pool(name="spool", bufs=3) as spool,
            tc.tile_pool(name="zpool", bufs=6, space="PSUM") as zpool,
            tc.tile_pool(name="zqpool", bufs=2, space="PSUM") as zqpool,
        ):
            # one-time constants
            w3t_s = consts.tile([H, m_loc + 31], F16)
            nc.sync.dma_start(out=w3t_s, in_=w3t16)
            b0t_s = consts.tile([H, m_loc], F32)
            nc.sync.dma_start(out=b0t_s, in_=b0t)
            b1t_s = consts.tile([H, m_loc], F32)
            nc.sync.dma_start(out=b1t_s, in_=b1t)
            b2t_s = consts.tile([H, m_loc], F32)
            nc.sync.dma_start(out=b2t_s, in_=b2t)
            b3t_s = consts.tile([GRP, ngrp], F32)
            nc.sync.dma_start(out=b3t_s, in_=b3t)
            hconst = None
            h16const = None
            if not do_pass:
                hconst = consts.tile([H, B], F32R)
                for q in range(B // 128):
                    nc.vector.tensor_copy(hconst[:, q * 128 : (q + 1) * 128], b0t_s[:, 0:128].bitcast(F32R))
                h16const = consts.tile([H, B], F16)
                nc.vector.tensor_copy(h16const, hconst)

            def relu_half(dst_half, z_half, bias_ap, on_act):
                # dst[128, HB] sbuf <- relu(z[128, HB] psum + bias)
                if on_act:
                    nc.scalar.activation(
                        dst_half, z_half, AF.Relu, bias=bias_ap, scale=1.0,
                    )
                else:
                    nc.vector.tensor_scalar(
                        out=dst_half, in0=z_half,
                        scalar1=bias_ap, scalar2=0.0, op0=OP.add, op1=OP.max,
                    )

            def body():
                w12s = xts = w0s = scrw = ygat = None

                def load_batches(m):
                    nonlocal w12s, xts, w0s, scrw
                    if m % W0GRP == 0:
                        w0s = w0pool.tile([DIN, W0GRP * H], F32R, tag="w0")
                        nc.sync.dma_start(
                            out=w0s,
                            in_=w0[m : m + W0GRP].rearrange("m i h -> i m h").bitcast(F32R),
                        )
                    if m % WGRP == 0:
                        w12s = wpool.tile([H, WGRP * 2 * H], F32R, tag="w12")
                        nc.sync.dma_start(
                            out=w12s,
                            in_=w12[m : m + WGRP].rearrange("m l h k -> h m l k").bitcast(F32R),
                        )
                    if m % XGRP == 0:
                        xts = xpool.tile([DIN, XGRP * B], F32R, tag="xt")
                        nc.sync.dma_start(
                            out=xts,
                            in_=xt[m : m + XGRP].rearrange("m i b -> i m b").bitcast(F32R),
                        )

                for g in range(ngrp):
                    g0 = g * GRP
                    ygat = ypool.tile([GRP, B], F32, tag="ygat")
                    rounds = [list(range(g0 + r, min(g0 + r + ILV, g0 + GRP)))
                              for r in range(0, GRP, ILV)]
                    for models in rounds:
                        ctx = {}
                        for m in models:
                            load_batches(m)
                            ctx[m] = (w12s, xts, w0s,
                                      (m % XGRP) * B, (m % WGRP) * 2 * H, (m % W0GRP) * H)

                        def mm_layer(lsel, rhs_of, zdict):
                            for m in models:
                                ws_, xs_, w0_, xo, wo, w0o = ctx[m]
                                za = zpool.tile([H, HB], F32, tag="z")
                                zb = zpool.tile([H, HB], F32, tag="z")
                                if lsel == 0:
                                    lhs = w0_[:, w0o : w0o + H]
                                    ra = xs_[:, xo : xo + HB]
                                    rb = xs_[:, xo + HB : xo + B]
                                else:
                                    lhs = ws_[:, wo + (lsel - 1) * H : wo + lsel * H]
                                    h = rhs_of[m]
                                    ra = h[:, 0:HB]
                                    rb = h[:, HB:B]
                                nc.tensor.matmul(za, lhs, ra, start=True, stop=True)
                                nc.tensor.matmul(zb, lhs, rb, start=True, stop=True)
                                zdict[m] = (za, zb)

                        def relu_layer(zdict, bias_t, tag, dt, hdict):
                            for m in models:
                                za, zb = zdict[m]
                                h = hpool.tile([H, B], dt, tag=tag)
                                relu_half(h[:, 0:HB], za, bias_t[:, m : m + 1], True)
                                relu_half(h[:, HB:B], zb, bias_t[:, m : m + 1], False)
                                hdict[m] = h

                        zs, hs = {}, {}
                        mm_layer(0, None, zs)
                        if do_pass:
                            relu_layer(zs, b0t_s, "h1", F32R, hs)
                        else:
                            hs = {m: hconst for m in models}
                        zs = {}
                        mm_layer(1, hs, zs)
                        h2s = {}
                        if do_pass:
                            relu_layer(zs, b1t_s, "h2", F32R, h2s)
                        else:
                            h2s = {m: hconst for m in models}
                        zs = {}
                        mm_layer(2, h2s, zs)
                        h3s = {}
                        if do_pass:
                            relu_layer(zs, b2t_s, "h3", F16, h3s)
                        else:
                            h3s = {m: h16const for m in models}

                        # col-tiled quad L3: model j -> psum rows 32j
                        zqa = zqpool.tile([128, HB], F32, tag="zq")
                        zqb = zqpool.tile([128, HB], F32, tag="zq")
                        for j, m in enumerate(models):
                            nc.tensor.matmul(
                                zqa[32 * j : 32 * j + 32, :],
                                w3t_s[:, m : m + 32], h3s[m][:, 0:HB],
                                start=True, stop=True, tile_position=(0, 32 * j),
                            )
                        for j, m in enumerate(models):
                            nc.tensor.matmul(
                                zqb[32 * j : 32 * j + 32, :],
                                w3t_s[:, m : m + 32], h3s[m][:, HB:B],
                                start=True, stop=True, tile_position=(0, 32 * j),
                            )
                        if do_pass:
                            scr = spool.tile([128, B], F32, tag="scr")
                            nc.scalar.copy(out=scr[:, 0:HB], in_=zqa)
                            nc.scalar.copy(out=scr[:, HB:B], in_=zqb)
                            mi0 = models[0] - g0
                            nq = len(models)
                            sv = scr.rearrange("(a p) b -> a p b", a=4)[0:nq, 0, :]
                            nc.sync.dma_start(out=ygat[mi0 : mi0 + nq, :], in_=sv)

                    # bias add + store for the group
                    yout = ypool.tile([GRP, B], F32, tag="yout")
                    if do_pass:
                        nc.scalar.add(yout[0:GRP], ygat[0:GRP], b3t_s[0:GRP, g : g + 1])
                    else:
                        nc.vector.memset(yout[0:GRP], 0.0)
                        nc.vector.memset(ygat[0:GRP, 0:1], 0.0)
                    nc.sync.dma_start(out=y[g0 : g0 + GRP, :], in_=yout[0:GRP])

            if loop_n > 1:
                with tc.For_i(0, loop_n, 1):
                    body()
            else:
                body()

    nc.compile()
    return nc


_NC_CACHE = {}


def _get_nc(m_loc):
    if m_loc not in _NC_CACHE:
        _NC_CACHE[m_loc] = build_nc(m_loc)
    return _NC_CACHE[m_loc]


def _prep_core_inputs(x, W0, b0, W1, b1, W2, b2, W3, b3, sl):
    m_loc = sl.stop - sl.start
    ngrp = m_loc // GRP
    xt = np.ascontiguousarray(np.transpose(x[sl], (0, 2, 1)))  # [m, DIN, B]
    w12 = np.ascontiguousarray(
        np.stack([W1[sl], W2[sl]], axis=1)  # [m, 2, H, H]
    )
    b3_pad = b3[sl, 0].astype(np.float32)
    return {
        "xt": xt,
        "w0": np.ascontiguousarray(W0[sl]),
        "w12": w12,
        "w3t16": np.ascontiguousarray(
            np.pad(W3[sl, :, 0], ((0, 31), (0, 0))).T.astype(np.float16)
        ),  # [H, m+31]
        "b0t": np.ascontiguousarray(b0[sl].T),
        "b1t": np.ascontiguousarray(b1[sl].T),
        "b2t": np.ascontiguousarray(b2[sl].T),
        "b3t": np.ascontiguousarray(b3_pad.reshape(ngrp, GRP).T),
    }


def kernel(x, W0, b0, W1, b1, W2, b2, W3, b3):
    x = np.asarray(x, dtype=np.float32)
    W0 = np.asarray(W0, np.float32); b0 = np.asarray(b0, np.float32)
    W1 = np.asarray(W1, np.float32); b1 = np.asarray(b1, np.float32)
    W2 = np.asarray(W2, np.float32); b2 = np.asarray(b2, np.float32)
    W3 = np.asarray(W3, np.float32); b3 = np.asarray(b3, np.float32)

    m_tot = x.shape[0]
    m_loc = m_tot // N_CORES
    nc = _get_nc(m_loc)
    in_maps = [
        _prep_core_inputs(x, W0, b0, W1, b1, W2, b2, W3, b3,
                          slice(c * m_loc, (c + 1) * m_loc))
        for c in range(N_CORES)
    ]
    res = run_bass_kernel_spmd(nc, in_maps, core_ids=list(range(N_CORES)))
    out = np.concatenate([r["y"] for r in res.results], axis=0)
    return out.reshape(m_tot, B, 1).astype(np.float32)


# revision 22
# speedup vs baseline: 1.1987x; 1.1471x over previous
"""TRN2 Bass kernel for nn_EnsemblePointNet: 1296 independent 4-layer MLPs.

Strategy: shard the model dim (1296 -> 162 per core) across 8 NeuronCores.
Per model, activations live transposed ([feature, batch]) so every layer is
one PE matmul with the stored weights as lhsT:
    z = W.T @ h   (lhsT=W [K,M], rhs=h_T [K,B])
fp32r matmuls (full PE rate, ~1.5e-4 rel err). Bias+ReLU fused into the
PSUM->SBUF eviction pass, split between the Scalar (ACT) and Vector (DVE)
engines. DMAs are batched across models (per-DMA fixed cost ~1us dominates
otherwise). The [1,B] layer-3 outputs are evicted to a partition-0 scratch
row shared by XGRP models, DMA'd into a 128-model gather tile, bias-added
there, and stored contiguously.
"""

import contextlib
import sys

sys.path.insert(0, "/opt/trn_rl_repo")

import numpy as np

import concourse.bass as bass
import concourse.mybir as mybir
import concourse.tile as tile
from concourse import bacc
from concourse.bass_utils import run_bass_kernel_spmd

F32 = mybir.dt.float32
F32R = mybir.dt.float32r
F16 = mybir.dt.float16
AF = mybir.ActivationFunctionType
OP = mybir.AluOpType

M_TOT = 1296
N_CORES = 8
M_LOC = M_TOT // N_CORES  # 162
B = 1024
DIN = 8
H = 128

# ACT is 1.2 GHz, DVE 0.96 GHz; split each relu pass proportionally.
ACT_COLS = 640  # of 1024
HB = 512  # psum bank width in fp32

WGRP = 3   # models per w12 load (w1+w2 packed, 128KB each)
XGRP = 6   # models per xt load / scr evict row
W0GRP = 18  # models per w0 load
GRP = 54   # models per output gather group
ILV = 4    # models interleaved per pipeline round (quad for col-tiled L3)


def build_nc(m_loc=M_LOC, loop_n=1, do_pass=True, do_mm=True, do_dma=True):
    assert m_loc % GRP == 0 and GRP % XGRP == 0 and GRP % WGRP == 0 and GRP % W0GRP == 0
    nc = bacc.Bacc("TRN2", target_bir_lowering=False, debug=False)
    xt = nc.dram_tensor("xt", [m_loc, DIN, B], F32, kind="ExternalInput").ap()
    w0 = nc.dram_tensor("w0", [m_loc, DIN, H], F32, kind="ExternalInput").ap()
    w12 = nc.dram_tensor("w12", [m_loc, 2, H, H], F32, kind="ExternalInput").ap()
    w3t16 = nc.dram_tensor("w3t16", [H, m_loc + 31], F16, kind="ExternalInput").ap()
    b0t = nc.dram_tensor("b0t", [H, m_loc], F32, kind="ExternalInput").ap()
    b1t = nc.dram_tensor("b1t", [H, m_loc], F32, kind="ExternalInput").ap()
    b2t = nc.dram_tensor("b2t", [H, m_loc], F32, kind="ExternalInput").ap()
    ngrp = m_loc // GRP
    b3t = nc.dram_tensor("b3t", [GRP, ngrp], F32, kind="ExternalInput").ap()
    y = nc.dram_tensor("y", [m_loc, B], F32, kind="ExternalOutput").ap()

    with tile.TileContext(nc) as tc:
        with (
            tc.tile_pool(name="consts", bufs=1) as consts,
            tc.tile_pool(name="wpool", bufs=4) as wpool,
            tc.tile_pool(name="w0pool", bufs=2) as w0pool,
            tc.tile_pool(name="xpool", bufs=3) as xpool,
            tc.tile_pool(name="hpool", bufs=5) as hpool,
            tc.tile_pool(name="ypool", bufs=2) as ypool,
            tc.tile_pool(name="spool", bufs=2) as spool,
            tc.tile_pool(name="zpool", bufs=6, space="PSUM") as zpool,
            tc.tile_pool(name="zqpool", bufs=2, space="PSUM") as zqpool,
        ):
            # one-time constants
            w3t_s = consts.tile([H, m_loc + 31], F16)
            nc.sync.dma_start(out=w3t_s, in_=w3t16)
            b0t_s = consts.tile([H, m_loc], F32)
            nc.sync.dma_start(out=b0t_s, in_=b0t)
            b1t_s = consts.tile([H, m_loc], F32)
            nc.sync.dma_start(out=b1t_s, in_=b1t)
            b2t_s = consts.tile([H, m_loc], F32)
            nc.sync.dma_start(out=b2t_s, in_=b2t)
            b3t_s = consts.tile([GRP, ngrp], F32)
            nc.sync.dma_start(out=b3t_s, in_=b3t)
            hconst = None
            h16const = None
            if not do_pass:
                hconst = consts.tile([H, B], F32R)
                for q in range(B // 128):
                    nc.vector.tensor_copy(hconst[:, q * 128 : (q + 1) * 128], b0t_s[:, 0:128].bitcast(F32R))
                h16const = consts.tile([H, B], F16)
                nc.vector.tensor_copy(h16const, hconst)

            def relu_half(dst_half, z_half, bias_ap, on_act):
                # dst[128, HB] sbuf <- relu(z[128, HB] psum + bias)
                if on_act:
                    nc.scalar.activation(
                        dst_half, z_half, AF.Relu, bias=bias_ap, scale=1.0,
                    )
                else:
                    nc.vector.tensor_scalar(
                        out=dst_half, in0=z_half,
                        scalar1=bias_ap, scalar2=0.0, op0=OP.add, op1=OP.max,
                    )

            def body():
                w12s = xts = w0s = scrw = ygat = None

                def load_batches(m):
                    nonlocal w12s, xts, w0s, scrw
                    if m % W0GRP == 0:
                        w0s = w0pool.tile([DIN, W0GRP * H], F32R, tag="w0")
                        nc.sync.dma_start(
                            out=w0s,
                            in_=w0[m : m + W0GRP].rearrange("m i h -> i m h").bitcast(F32R),
                        )
                    if m % WGRP == 0:
                        w12s = wpool.tile([H, WGRP * 2 * H], F32R, tag="w12")
                        nc.sync.dma_start(
                            out=w12s,
                            in_=w12[m : m + WGRP].rearrange("m l h k -> h m l k").bitcast(F32R),
                        )
                    if m % XGRP == 0:
                        xts = xpool.tile([DIN, XGRP * B], F32R, tag="xt")
                        nc.sync.dma_start(
                            out=xts,
                            in_=xt[m : m + XGRP].rearrange("m i b -> i m b").bitcast(F32R),
                        )

                for g in range(ngrp):
                    g0 = g * GRP
                    ygat = ypool.tile([GRP, B], F32, tag="ygat")
                    rounds = [list(range(g0 + r, min(g0 + r + ILV, g0 + GRP)))
                              for r in range(0, GRP, ILV)]
                    for models in rounds:
                        ctx = {}
                        for m in models:
                            load_batches(m)
                            ctx[m] = (w12s, xts, w0s,
                                      (m % XGRP) * B, (m % WGRP) * 2 * H, (m % W0GRP) * H)

                        def mm_layer(lsel, rhs_of, zdict):
                            for m in models:
                                ws_, xs_, w0_, xo, wo, w0o = ctx[m]
                                za = zpool.tile([H, HB], F32, tag="z")
                                zb = zpool.tile([H, HB], F32, tag="z")
                                if lsel == 0:
                                    lhs = w0_[:, w0o : w0o + H]
                                    ra = xs_[:, xo : xo + HB]
                                    rb = xs_[:, xo + HB : xo + B]
                                else:
                                    lhs = ws_[:, wo + (lsel - 1) * H : wo + lsel * H]
                                    h = rhs_of[m]
                                    ra = h[:, 0:HB]
                                    rb = h[:, HB:B]
                                nc.tensor.matmul(za, lhs, ra, start=True, stop=True)
                                nc.tensor.matmul(zb, lhs, rb, start=True, stop=True)
                                zdict[m] = (za, zb)

                        def relu_layer(zdict, bias_t, tag, dt, hdict):
                            for m in models:
                                za, zb = zdict[m]
                                h = hpool.tile([H, B], dt, tag=tag)
                                relu_half(h[:, 0:HB], za, bias_t[:, m : m + 1], True)
                                relu_half(h[:, HB:B], zb, bias_t[:, m : m + 1], False)
                                hdict[m] = h

                        zs, hs = {}, {}
                        mm_layer(0, None, zs)
                        if do_pass:
                            relu_layer(zs, b0t_s, "h1", F32R, hs)
                        else:
                            hs = {m: hconst for m in models}
                        zs = {}
                        mm_layer(1, hs, zs)
                        h2s = {}
                        if do_pass:
                            relu_layer(zs, b1t_s, "h2", F32R, h2s)
                        else:
                            h2s = {m: hconst for m in models}
                        zs = {}
                        mm_layer(2, h2s, zs)
                        h3s = {}
                        if do_pass:
                            relu_layer(zs, b2t_s, "h3", F16, h3s)
                        else:
                            h3s = {m: h16const for m in models}

                        # col-tiled quad L3: model j -> psum rows 32j
                        zqa = zqpool.tile([128, HB], F32, tag="zq")
                        zqb = zqpool.tile([128, HB], F32, tag="zq")
                        for j, m in enumerate(models):
                            nc.tensor.matmul(
                                zqa[32 * j : 32 * j + 32, :],
                                w3t_s[:, m : m + 32], h3s[m][:, 0:HB],
                                start=True, stop=True, tile_position=(0, 32 * j),
                            )
                            nc.tensor.matmul(
                                zqb[32 * j : 32 * j + 32, :],
                                w3t_s[:, m : m + 32], h3s[m][:, HB:B],
                                start=True, stop=True, tile_position=(0, 32 * j),
                            )
                        if do_pass:
                            scr = spool.tile([128, B], F32, tag="scr")
                            nc.scalar.copy(out=scr[:, 0:HB], in_=zqa)
                            nc.scalar.copy(out=scr[:, HB:B], in_=zqb)
                            mi0 = models[0] - g0
                            nq = len(models)
                            sv = scr.rearrange("(a p) b -> a p b", a=4)[0:nq, 0, :]
                            nc.sync.dma_start(out=ygat[mi0 : mi0 + nq, :], in_=sv)

                    # bias add + store for the group
                    yout = ypool.tile([GRP, B], F32, tag="yout")
                    if do_pass:
                        nc.scalar.add(yout[0:GRP], ygat[0:GRP], b3t_s[0:GRP, g : g + 1])
                    else:
                        nc.vector.memset(yout[0:GRP], 0.0)
                        nc.vector.memset(ygat[0:GRP, 0:1], 0.0)
                    nc.sync.dma_start(out=y[g0 : g0 + GRP, :], in_=yout[0:GRP])

            if loop_n > 1:
                with tc.For_i(0, loop_n, 1):
                    body()
            else:
                body()

    nc.compile()
    return nc


_NC_CACHE = {}


def _get_nc(m_loc):
    if m_loc not in _NC_CACHE:
        _NC_CACHE[m_loc] = build_nc(m_loc)
    return _NC_CACHE[m_loc]


def _prep_core_inputs(x, W0, b0, W1, b1, W2, b2, W3, b3, sl):
    m_loc = sl.stop - sl.start
    ngrp = m_loc // GRP
    xt = np.ascontiguousarray(np.transpose(x[sl], (0, 2, 1)))  # [m, DIN, B]
    w12 = np.ascontiguousarray(
        np.stack([W1[sl], W2[sl]], axis=1)  # [m, 2, H, H]
    )
    b3_pad = b3[sl, 0].astype(np.float32)
    return {
        "xt": xt,
        "w0": np.ascontiguousarray(W0[sl]),
        "w12": w12,
        "w3t16": np.ascontiguousarray(
            np.pad(W3[sl, :, 0], ((0, 31), (0, 0))).T.astype(np.float16)
        ),  # [H, m+31]
        "b0t": np.ascontiguousarray(b0[sl].T),
        "b1t": np.ascontiguousarray(b1[sl].T),
        "b2t": np.ascontiguousarray(b2[sl].T),
        "b3t": np.ascontiguousarray(b3_pad.reshape(ngrp, GRP).T),
    }


def kernel(x, W0, b0, W1, b1, W2, b2, W3, b3):
    x = np.asarray(x, dtype=np.float32)
    W0 = np.asarray(W0, np.float32); b0 = np.asarray(b0, np.float32)
    W1 = np.asarray(W1, np.float32); b1 = np.asarray(b1, np.float32)
    W2 = np.asarray(W2, np.float32); b2 = np.asarray(b2, np.float32)
    W3 = np.asarray(W3, np.float32); b3 = np.asarray(b3, np.float32)

    m_tot = x.shape[0]
    m_loc = m_tot // N_CORES
    nc = _get_nc(m_loc)
    in_maps = [
        _prep_core_inputs(x, W0, b0, W1, b1, W2, b2, W3, b3,
                          slice(c * m_loc, (c + 1) * m_loc))
        for c in range(N_CORES)
    ]
    res = run_bass_kernel_spmd(nc, in_maps, core_ids=list(range(N_CORES)))
    out = np.concatenate([r["y"] for r in res.results], axis=0)
    return out.reshape(m_tot, B, 1).astype(np.float32)
